# revision 14
# baseline (speedup 1.0000x reference)
"""Cell-list pairwise distance screen (CellList) for 8 Trainium2 NeuronCores.

Computes the masked dense [N, N] lower-triangular distance matrix:
  out[i, j] = sqrt(|c_i - c_j|^2)  if  j < i, both species valid, d2 <= cutoff^2
            = 0                    otherwise

Strategy (block-sparse + single-matmul d2):
  - Host sorts atoms along a Hilbert curve over 2.5 A cells -> 48 row blocks
    of 128 spatially-compact atoms.  For each row block R it gathers the
    candidate columns {j : dist(j, bbox_R) <= cutoff, block(j) <= R} -- a
    conservative superset of all pairs, deduplicated at block level (each
    cross-block pair appears in exactly one list; own-block pairs appear in
    both orientations of the self tile and scatter to the same output slot).
  - d2 is produced by ONE tensor-engine matmul per 512-col piece:
      d2[i,j] = ri + rj - 2*ci.cj
    expanded over exact 3-way bf16 splits of the per-block-translated
    coordinates (local coords ~ +-13 A, so f32 cancellation error ~1e-5,
    far below the ~3e-5 spacing of d2 values near cutoff^2; measured 0 mask
    flips vs the f32 reference on the target data).  33 contraction rows per
    block: 27 split cross products + 3 ri splits (x ones) + 3 rj splits.
  - Up to 3 row-block segments pack into one 512-col piece as block-diagonal
    bands (K = 99): a column's rhs rows are zero outside its own band, so
    each column only accumulates its own block's terms.  Candidate lists
    split freely across pieces/cores, so the 8 cores get an equal number of
    nearly-full pieces.
  - PSUM then holds d2 directly: one DVE bandpass (select t in (1e-3,
    cutoff^2], else 0) -> one ACT sqrt to fp16 -> DMA out.  Host scatters
    the compacted fp16 values into the full [N, N] f32 zero matrix.
  - All DRAM tensors are laid out so every per-piece DMA is one contiguous
    block (cheap descriptors): rhs [NP*K, W], wts [NP*K, P], out [NP*P, W].
"""

import threading

import numpy as np

N = 6144
P = 128
NCORES = 8
W = 512  # piece width (one PSUM bank)
KB = 33  # contraction rows per band
BANDS = 3  # bands (row-block segments) per piece
K = KB * BANDS  # 99

_lock = threading.Lock()
_cache: dict = {}


def _register_ops():
    """Register the fused DVE bandpass op at runtime (visible to table-gen)."""
    import concourse.dve_ops as dve_ops
    from concourse.dve_spec import (
        C0,
        C1,
        Spec,
        Src0,
        Zero,
        _has_src1,
        lower,
        select,
    )
    from concourse.dve_uop import DveOpSpec

    def make(name, body, ref):
        for op in dve_ops.OPS:
            if op.name == name:
                return op
        spec = Spec(body=body, reference=ref)
        row = 1 + len(dve_ops.OPS)
        assert row < 0x20
        shas = {}
        for ver in ("v3", "v4"):
            uops = lower(spec, ver=ver)
            shas[ver] = DveOpSpec(
                name=name, opcode=row, uops=uops, rd1_en=_has_src1(spec)
            ).sha(ver)
        op = dve_ops.DveOp(name, spec, subdim=False, uops_sha=shas)
        dve_ops._SUB_OPCODE_FOR_NAME[name] = row
        dve_ops.OPS.append(op)
        dve_ops.CUSTOM_DVE_SPECS[name] = spec
        return op

    # out = (s0 < in0 < s1) ? in0 : 0
    def band_ref(in0, in1, s0, s1, imm2):
        t = in0.astype(np.float32)
        keep = (t > s0) & (t < s1)
        return np.where(keep, t, 0.0).astype(np.float32)

    bandpass = make(
        "BANDPASS_ANT",
        select((Src0 > C0) & (Src0 < C1), Src0, Zero),
        band_ref,
    )
    return bandpass


def _build_program(NP, cuthi):
    import concourse.bacc as bacc
    import concourse.mybir as mybir
    import concourse.tile as tile

    bandpass = _register_ops()

    nc = bacc.Bacc("TRN2", target_bir_lowering=False, debug=False, num_devices=NCORES)
    f32 = mybir.dt.float32
    f16 = mybir.dt.float16
    bf16 = mybir.dt.bfloat16
    PW = P + W  # fused [wts | rhs] block width per piece

    NSP = (NP + 1) // 2  # 1024-col super-pieces for DVE/ACT/out-DMA
    inp = nc.dram_tensor("inp", [K, NP * PW], bf16, kind="ExternalInput")
    out = nc.dram_tensor("out", [NSP * P, 2 * W], f16, kind="ExternalOutput")

    with tile.TileContext(nc) as tc:
        with (
            tc.tile_pool(name="const", bufs=1) as cpool,
            tc.tile_pool(name="work", bufs=3) as wpool,
            tc.tile_pool(name="outp", bufs=3) as spool,
            tc.tile_pool(name="psx", bufs=3, space="PSUM") as ppx,
        ):
            cc_t = cpool.tile([P, 2], f32, tag="cc")
            inpA = cpool.tile([K, PW], bf16, tag="inpA")
            inpB = cpool.tile([K, (NP - 1) * PW], bf16, tag="inpB")
            warm_t = cpool.tile([P, 2], f32, tag="warm")

            # just TWO input DMAs: piece 0 alone (fast first matmul) and
            # the rest as one bulk transfer -- per-DMA launch latency
            # (~2.4us) dwarfs bandwidth here, so fewer DMAs win.
            nc.sync.dma_start(inpA[:], inp[:, 0:PW])
            nc.gpsimd.dma_start(inpB[:], inp[:, PW : NP * PW])

            # bandpass consts via memset on the otherwise-idle vector queue
            # (no DMA); cuthi is baked per build
            nc.vector.memset(cc_t[:, 0:1], 1e-3)
            nc.vector.memset(cc_t[:, 1:2], float(cuthi))

            # pull the ACT sqrt table in
            nc.vector.memset(warm_t[:, 0:1], 1.0)
            nc.scalar.sqrt(warm_t[:, 1:2], warm_t[:, 0:1])

            def piece_aps(p):
                if p == 0:
                    return inpA[:, 0:P], inpA[:, P:PW]
                base = (p - 1) * PW
                return inpB[:, base : base + P], inpB[:, base + P : base + PW]

            for sp in range(NSP):
                pieces = [p for p in (2 * sp, 2 * sp + 1) if p < NP]
                w = len(pieces) * W
                t = ppx.tile([P, 2 * W], f32, tag="t")
                for h, p in enumerate(pieces):
                    lhsT, rhs = piece_aps(p)
                    nc.tensor.matmul(
                        t[:, h * W : (h + 1) * W],
                        lhsT,
                        rhs,
                        start=True,
                        stop=True,
                    )
                v = wpool.tile([P, 2 * W], f16, tag="v")
                nc.vector._custom_dve(
                    bandpass,
                    out=v[:, 0:w],
                    in0=t[:, 0:w],
                    s0=cc_t[:, 0:1],
                    s1=cc_t[:, 1:2],
                )
                s = spool.tile([P, 2 * W], f16, tag="s")
                nc.scalar.sqrt(s[:, 0:w], v[:, 0:w])
                q = nc.gpsimd if sp % 2 == 0 else nc.sync
                q.dma_start(out[sp * P : (sp + 1) * P, 0:w], s[:, 0:w])

    nc.compile()
    return nc


def _get_program(NP, cuthi):
    with _lock:
        key = f"nc{NP}-{float(cuthi)}"
        if key not in _cache:
            _cache[key] = _build_program(NP, cuthi)
    return _cache[key]


def _hilbert_sort(coords):
    """Atom permutation along a Hilbert curve over a 16^3 grid."""
    lo = coords.min(0)
    ext = np.maximum(coords.max(0) - lo, 1e-6)
    cell = np.clip((coords - lo) / ext * 16.0, 0, 15.999).astype(np.int64)
    X = cell.T.astype(np.uint64).copy()
    n, bits = 3, 4
    M = np.uint64(1) << np.uint64(bits - 1)
    Q = M
    while Q > np.uint64(1):
        Pm = Q - np.uint64(1)
        for i in range(n):
            hi = (X[i] & Q) != 0
            X[0] = np.where(hi, X[0] ^ Pm, X[0])
            t = (X[0] ^ X[i]) & Pm
            X[0] = np.where(hi, X[0], X[0] ^ t)
            X[i] = np.where(hi, X[i], X[i] ^ t)
        Q >>= np.uint64(1)
    for i in range(1, n):
        X[i] ^= X[i - 1]
    t = np.zeros(len(cell), np.uint64)
    Q = M
    while Q > np.uint64(1):
        t = np.where((X[n - 1] & Q) != 0, t ^ (Q - np.uint64(1)), t)
        Q >>= np.uint64(1)
    for i in range(n):
        X[i] ^= t
    key = np.zeros(len(cell), np.int64)
    for b in range(bits):
        for i in range(3):
            key |= np.int64(((X[i] >> np.uint64(b)) & np.uint64(1)).astype(np.int64)) << np.int64(
                3 * b + (2 - i)
            )
    return np.argsort(key, kind="stable")


def _split3(v32):
    """Exact 3-way bf16 split: v32 == hi + mid + lo (as f32 sums)."""
    import ml_dtypes

    bf = ml_dtypes.bfloat16
    hi = v32.astype(bf)
    r1 = (v32 - hi.astype(np.float32)).astype(np.float32)
    mid = r1.astype(bf)
    r2 = (r1 - mid.astype(np.float32)).astype(np.float32)
    lo = r2.astype(bf)
    recon = (
        hi.astype(np.float32) + mid.astype(np.float32) + lo.astype(np.float32)
    ).astype(np.float32)
    assert np.array_equal(recon, v32), "bf16 3-way split not exact"
    return hi.astype(np.float32), mid.astype(np.float32), lo.astype(np.float32)


def _prepare(species, coordinates, cutoff):
    """Build per-core in_maps plus host-side scatter indices."""
    import ml_dtypes

    bf = ml_dtypes.bfloat16
    coords = np.asarray(coordinates, dtype=np.float32).reshape(-1, 3).copy()
    n = coords.shape[0]
    assert n == N and n % P == 0, coords.shape
    valid = np.asarray(species).reshape(-1) >= 0
    if not valid.all():
        bad = np.where(~valid)[0]
        far = float(coords[valid].max()) if valid.any() else 0.0
        coords[bad] = (far + 20.0 + 10.0 * np.arange(len(bad), dtype=np.float32))[
            :, None
        ]

    cutf = float(cutoff)
    cut2 = np.float32(cutf) * np.float32(cutf)
    cuthi = np.nextafter(cut2, np.float32(np.inf), dtype=np.float32)
    prune2 = (cutf + 1e-3) ** 2  # conservative host-side pruning radius

    pi = _hilbert_sort(coords)
    cs = coords[pi].astype(np.float32)
    NB = n // P
    blk = np.arange(n) // P

    # candidate columns per row block, deduped at block level
    cands = []
    for R in range(NB):
        rows = cs[R * P : (R + 1) * P]
        bmin, bmax = rows.min(0), rows.max(0)
        d = np.maximum(0, np.maximum(bmin[None, :] - cs, cs - bmax[None, :]))
        cand = np.where(((d * d).sum(1) <= prune2) & (blk <= R))[0]
        cands.append(cand)

    # greedy pour, largest list first, splitting freely at piece boundaries;
    # each piece holds <= W cols and <= BANDS row-block segments
    order = sorted(range(NB), key=lambda R: -len(cands[R]))
    bins = []  # [space_left, [(R, start, width), ...]]
    cur = None
    for R in order:
        left = len(cands[R])
        s0 = 0
        while left > 0:
            if cur is None or cur[0] == 0 or len(cur[1]) == BANDS:
                bins.append([W, []])
                cur = bins[-1]
            take = min(left, cur[0])
            cur[1].append((R, s0, take))
            cur[0] -= take
            s0 += take
            left -= take
    nbins = len(bins)
    NP = max(1, -(-nbins // NCORES))

    # assign bins to cores round-robin by size
    border = sorted(range(nbins), key=lambda i: -(W - bins[i][0]))
    per_core = [[] for _ in range(NCORES)]
    for i, b in enumerate(border):
        per_core[i % NCORES].append(bins[b])

    in_maps = []
    idx_maps = []
    for c in range(NCORES):
        wts_m = np.zeros((NP * K, P), np.float32)
        rhs_m = np.zeros((NP * K, W), np.float32)
        idx_m = np.full(((NP + 1) // 2, P, 2 * W), N * N, np.int64)
        for p, (_, chlist) in enumerate(per_core[c]):
            off = 0
            for band, (R, s0, w) in enumerate(chlist):
                rows = cs[R * P : (R + 1) * P]
                bmin, bmax = rows.min(0), rows.max(0)
                tR = ((bmin + bmax) * np.float32(0.5)).astype(np.float32)
                rl = (rows - tR).astype(np.float32)
                cand = cands[R][s0 : s0 + w]
                cl = (cs[cand] - tR).astype(np.float32)
                ri = ((rl[:, 0] * rl[:, 0] + rl[:, 1] * rl[:, 1]) + rl[:, 2] * rl[:, 2]).astype(np.float32)
                rj = ((cl[:, 0] * cl[:, 0] + cl[:, 1] * cl[:, 1]) + cl[:, 2] * cl[:, 2]).astype(np.float32)
                kb = p * K + band * KB
                rcol = slice(off, off + w)
                for ci in range(3):
                    rs = _split3(rl[:, ci].copy())
                    csp = _split3(cl[:, ci].copy())
                    for a in range(3):
                        wa = (np.float32(-2.0) * rs[a]).astype(bf).astype(np.float32)
                        for bb in range(3):
                            row = kb + ci * 9 + a * 3 + bb
                            wts_m[row, :] = wa
                            rhs_m[row, rcol] = csp[bb]
                for a, sp in enumerate(_split3(ri.copy())):
                    wts_m[kb + 27 + a, :] = sp
                    rhs_m[kb + 27 + a, rcol] = 1.0
                for bb, sp in enumerate(_split3(rj.copy())):
                    wts_m[kb + 30 + bb, :] = 1.0
                    rhs_m[kb + 30 + bb, rcol] = sp
                # scatter indices: orig (hi, lo) pair -> tril slot; self -> scratch
                ro = pi[R * P : (R + 1) * P]
                co = pi[cand]
                hi = np.maximum(ro[:, None], co[None, :])
                lo = np.minimum(ro[:, None], co[None, :])
                flat = hi * N + lo
                flat[ro[:, None] == co[None, :]] = N * N
                cb = (p % 2) * W
                idx_m[p // 2, :, cb + off : cb + off + w] = flat
                off += w
        # [K, NP*(P+W)]: piece p at cols [p*PW, (p+1)*PW), wts then rhs
        inp_m = (
            np.concatenate(
                [wts_m.reshape(NP, K, P), rhs_m.reshape(NP, K, W)], axis=2
            )
            .transpose(1, 0, 2)
            .reshape(K, NP * (P + W))
        )
        in_maps.append({"inp": np.ascontiguousarray(inp_m).astype(bf)})
        idx_maps.append(idx_m)
    with _lock:
        _cache["cuthi"] = float(cuthi)
    return in_maps, idx_maps, NP


def _prepare_inputs(species, coordinates, cutoff):
    in_maps, idx_maps, NP = _prepare(species, coordinates, cutoff)
    return in_maps


def _run(in_maps, trace=False):
    from concourse import bass_utils

    NP = in_maps[0]["inp"].shape[1] // (P + W)
    with _lock:
        cuthi = _cache["cuthi"]
    nc = _get_program(NP, cuthi)
    return bass_utils.run_bass_kernel_spmd(
        nc, in_maps, core_ids=list(range(NCORES)), trace=trace
    )


def _assemble(results, idx_maps):
    full = np.zeros(N * N + 1, np.float32)
    for c in range(NCORES):
        vals = results[c]["out"].astype(np.float32)
        full[idx_maps[c].ravel()] = vals.ravel()
    return full[: N * N].reshape(N, N)


def kernel(species, coordinates, cutoff):
    in_maps, idx_maps, NP = _prepare(species, coordinates, cutoff)
    res = _run(in_maps)
    return _assemble(res.results, idx_maps)


# revision 19
# speedup vs baseline: 1.0326x; 1.0326x over previous
"""Cell-list pairwise distance screen (CellList) for 8 Trainium2 NeuronCores.

Computes the masked dense [N, N] lower-triangular distance matrix:
  out[i, j] = sqrt(|c_i - c_j|^2)  if  j < i, both species valid, d2 <= cutoff^2
            = 0                    otherwise

Strategy (block-sparse + single-matmul d2):
  - Host sorts atoms along a Hilbert curve over 2.5 A cells -> 48 row blocks
    of 128 spatially-compact atoms.  For each row block R it gathers the
    candidate columns {j : dist(j, bbox_R) <= cutoff, block(j) <= R} -- a
    conservative superset of all pairs, deduplicated at block level (each
    cross-block pair appears in exactly one list; own-block pairs appear in
    both orientations of the self tile and scatter to the same output slot).
  - d2 is produced by ONE tensor-engine matmul per 512-col piece:
      d2[i,j] = ri + rj - 2*ci.cj
    expanded over exact 3-way bf16 splits of the per-block-translated
    coordinates (local coords ~ +-13 A, so f32 cancellation error ~1e-5,
    far below the ~3e-5 spacing of d2 values near cutoff^2; measured 0 mask
    flips vs the f32 reference on the target data).  33 contraction rows per
    block: 27 split cross products + 3 ri splits (x ones) + 3 rj splits.
  - Up to 3 row-block segments pack into one 512-col piece as block-diagonal
    bands (K = 99): a column's rhs rows are zero outside its own band, so
    each column only accumulates its own block's terms.  Candidate lists
    split freely across pieces/cores, so the 8 cores get an equal number of
    nearly-full pieces.
  - PSUM then holds d2 directly: one DVE bandpass (select t in (1e-3,
    cutoff^2], else 0) -> one ACT sqrt to fp16 -> DMA out.  Host scatters
    the compacted fp16 values into the full [N, N] f32 zero matrix.
  - All DRAM tensors are laid out so every per-piece DMA is one contiguous
    block (cheap descriptors): rhs [NP*K, W], wts [NP*K, P], out [NP*P, W].
"""

import threading

import numpy as np

N = 6144
P = 128
NCORES = 8
W = 512  # piece width (one PSUM bank)
KB = 24  # contraction rows per band
BANDS = 3  # bands (row-block segments) per piece
K = KB * BANDS  # 72
# cross-product split pairs kept (row-split a, col-split b); the dropped
# (m,l)/(l,m)/(l,l) terms are < 1e-5 in d2 -- measured 0 mask flips
KEEP = [(0, 0), (0, 1), (0, 2), (1, 0), (1, 1), (2, 0)]

_lock = threading.Lock()
_cache: dict = {}


def _register_ops():
    """Register the fused DVE bandpass op at runtime (visible to table-gen)."""
    import concourse.dve_ops as dve_ops
    from concourse.dve_spec import (
        C0,
        C1,
        Spec,
        Src0,
        Zero,
        _has_src1,
        lower,
        select,
    )
    from concourse.dve_uop import DveOpSpec

    def make(name, body, ref):
        for op in dve_ops.OPS:
            if op.name == name:
                return op
        spec = Spec(body=body, reference=ref)
        row = 1 + len(dve_ops.OPS)
        assert row < 0x20
        shas = {}
        for ver in ("v3", "v4"):
            uops = lower(spec, ver=ver)
            shas[ver] = DveOpSpec(
                name=name, opcode=row, uops=uops, rd1_en=_has_src1(spec)
            ).sha(ver)
        op = dve_ops.DveOp(name, spec, subdim=False, uops_sha=shas)
        dve_ops._SUB_OPCODE_FOR_NAME[name] = row
        dve_ops.OPS.append(op)
        dve_ops.CUSTOM_DVE_SPECS[name] = spec
        return op

    # out = (s0 < in0 < s1) ? in0 : 0
    def band_ref(in0, in1, s0, s1, imm2):
        t = in0.astype(np.float32)
        keep = (t > s0) & (t < s1)
        return np.where(keep, t, 0.0).astype(np.float32)

    bandpass = make(
        "BANDPASS_ANT",
        select((Src0 > C0) & (Src0 < C1), Src0, Zero),
        band_ref,
    )
    return bandpass


def _build_program(NP, cuthi):
    import concourse.bacc as bacc
    import concourse.mybir as mybir
    import concourse.tile as tile

    bandpass = _register_ops()

    nc = bacc.Bacc("TRN2", target_bir_lowering=False, debug=False, num_devices=NCORES)
    f32 = mybir.dt.float32
    f16 = mybir.dt.float16
    bf16 = mybir.dt.bfloat16
    PW = P + W  # fused [wts | rhs] block width per piece

    NSP = (NP + 1) // 2  # 1024-col super-pieces for DVE/ACT/out-DMA
    inp = nc.dram_tensor("inp", [NP * K, PW], bf16, kind="ExternalInput")
    out = nc.dram_tensor("out", [NSP * P, 2 * W], f16, kind="ExternalOutput")

    with tile.TileContext(nc) as tc:
        with (
            tc.tile_pool(name="const", bufs=1) as cpool,
            tc.tile_pool(name="work", bufs=3) as wpool,
            tc.tile_pool(name="outp", bufs=3) as spool,
            tc.tile_pool(name="psx", bufs=3, space="PSUM") as ppx,
        ):
            cc_t = cpool.tile([P, 2], f32, tag="cc")
            inp_t = [
                cpool.tile([K, PW], bf16, tag=f"inp{p}", name=f"inp{p}")
                for p in range(NP)
            ]
            warm_t = cpool.tile([P, 2], f32, tag="warm")

            # one contiguous DMA per piece over all three hw issue queues;
            # per-DMA completion latency is ~2us, so earliest-needed first
            queues = [nc.sync, nc.gpsimd, nc.scalar, nc.sync, nc.gpsimd]
            for p in range(NP):
                queues[p % 5].dma_start(inp_t[p][:], inp[p * K : (p + 1) * K, :])

            # bandpass consts via memset on the otherwise-idle vector queue
            # (no DMA); cuthi is baked per build
            nc.vector.memset(cc_t[:, 0:1], 1e-3)
            nc.vector.memset(cc_t[:, 1:2], float(cuthi))

            # pull the ACT sqrt table in
            nc.vector.memset(warm_t[:, 0:1], 1.0)
            nc.scalar.sqrt(warm_t[:, 1:2], warm_t[:, 0:1])

            for sp in range(NSP):
                pieces = [p for p in (2 * sp, 2 * sp + 1) if p < NP]
                w = len(pieces) * W
                t = ppx.tile([P, 2 * W], f32, tag="t")
                for h, p in enumerate(pieces):
                    nc.tensor.matmul(
                        t[:, h * W : (h + 1) * W],
                        inp_t[p][:, 0:P],
                        inp_t[p][:, P:PW],
                        start=True,
                        stop=True,
                    )
                v = wpool.tile([P, 2 * W], f16, tag="v")
                nc.vector._custom_dve(
                    bandpass,
                    out=v[:, 0:w],
                    in0=t[:, 0:w],
                    s0=cc_t[:, 0:1],
                    s1=cc_t[:, 1:2],
                )
                s = spool.tile([P, 2 * W], f16, tag="s")
                nc.scalar.sqrt(s[:, 0:w], v[:, 0:w])
                q = nc.gpsimd if sp % 2 == 0 else nc.sync
                q.dma_start(out[sp * P : (sp + 1) * P, 0:w], s[:, 0:w])

    nc.compile()
    return nc


def _get_program(NP, cuthi):
    with _lock:
        key = f"nc{NP}-{float(cuthi)}"
        if key not in _cache:
            _cache[key] = _build_program(NP, cuthi)
    return _cache[key]


def _hilbert_sort(coords):
    """Atom permutation along a Hilbert curve over a 16^3 grid."""
    lo = coords.min(0)
    ext = np.maximum(coords.max(0) - lo, 1e-6)
    cell = np.clip((coords - lo) / ext * 16.0, 0, 15.999).astype(np.int64)
    X = cell.T.astype(np.uint64).copy()
    n, bits = 3, 4
    M = np.uint64(1) << np.uint64(bits - 1)
    Q = M
    while Q > np.uint64(1):
        Pm = Q - np.uint64(1)
        for i in range(n):
            hi = (X[i] & Q) != 0
            X[0] = np.where(hi, X[0] ^ Pm, X[0])
            t = (X[0] ^ X[i]) & Pm
            X[0] = np.where(hi, X[0], X[0] ^ t)
            X[i] = np.where(hi, X[i], X[i] ^ t)
        Q >>= np.uint64(1)
    for i in range(1, n):
        X[i] ^= X[i - 1]
    t = np.zeros(len(cell), np.uint64)
    Q = M
    while Q > np.uint64(1):
        t = np.where((X[n - 1] & Q) != 0, t ^ (Q - np.uint64(1)), t)
        Q >>= np.uint64(1)
    for i in range(n):
        X[i] ^= t
    key = np.zeros(len(cell), np.int64)
    for b in range(bits):
        for i in range(3):
            key |= np.int64(((X[i] >> np.uint64(b)) & np.uint64(1)).astype(np.int64)) << np.int64(
                3 * b + (2 - i)
            )
    return np.argsort(key, kind="stable")


def _split3(v32):
    """Exact 3-way bf16 split: v32 == hi + mid + lo (as f32 sums)."""
    import ml_dtypes

    bf = ml_dtypes.bfloat16
    hi = v32.astype(bf)
    r1 = (v32 - hi.astype(np.float32)).astype(np.float32)
    mid = r1.astype(bf)
    r2 = (r1 - mid.astype(np.float32)).astype(np.float32)
    lo = r2.astype(bf)
    recon = (
        hi.astype(np.float32) + mid.astype(np.float32) + lo.astype(np.float32)
    ).astype(np.float32)
    assert np.array_equal(recon, v32), "bf16 3-way split not exact"
    return hi.astype(np.float32), mid.astype(np.float32), lo.astype(np.float32)


def _prepare(species, coordinates, cutoff):
    """Build per-core in_maps plus host-side scatter indices."""
    import ml_dtypes

    bf = ml_dtypes.bfloat16
    coords = np.asarray(coordinates, dtype=np.float32).reshape(-1, 3).copy()
    n = coords.shape[0]
    assert n == N and n % P == 0, coords.shape
    valid = np.asarray(species).reshape(-1) >= 0
    if not valid.all():
        bad = np.where(~valid)[0]
        far = float(coords[valid].max()) if valid.any() else 0.0
        coords[bad] = (far + 20.0 + 10.0 * np.arange(len(bad), dtype=np.float32))[
            :, None
        ]

    cutf = float(cutoff)
    cut2 = np.float32(cutf) * np.float32(cutf)
    cuthi = np.nextafter(cut2, np.float32(np.inf), dtype=np.float32)
    prune2 = (cutf + 1e-3) ** 2  # conservative host-side pruning radius

    pi = _hilbert_sort(coords)
    cs = coords[pi].astype(np.float32)
    NB = n // P
    blk = np.arange(n) // P

    # candidate columns per row block, deduped at block level
    cands = []
    for R in range(NB):
        rows = cs[R * P : (R + 1) * P]
        bmin, bmax = rows.min(0), rows.max(0)
        d = np.maximum(0, np.maximum(bmin[None, :] - cs, cs - bmax[None, :]))
        cand = np.where(((d * d).sum(1) <= prune2) & (blk <= R))[0]
        cands.append(cand)

    # greedy pour, largest list first, splitting freely at piece boundaries;
    # each piece holds <= W cols and <= BANDS row-block segments
    order = sorted(range(NB), key=lambda R: -len(cands[R]))
    bins = []  # [space_left, [(R, start, width), ...]]
    cur = None
    for R in order:
        left = len(cands[R])
        s0 = 0
        while left > 0:
            if cur is None or cur[0] == 0 or len(cur[1]) == BANDS:
                bins.append([W, []])
                cur = bins[-1]
            take = min(left, cur[0])
            cur[1].append((R, s0, take))
            cur[0] -= take
            s0 += take
            left -= take
    nbins = len(bins)
    NP = max(1, -(-nbins // NCORES))

    # assign bins to cores round-robin by size
    border = sorted(range(nbins), key=lambda i: -(W - bins[i][0]))
    per_core = [[] for _ in range(NCORES)]
    for i, b in enumerate(border):
        per_core[i % NCORES].append(bins[b])

    in_maps = []
    idx_maps = []
    for c in range(NCORES):
        wts_m = np.zeros((NP * K, P), np.float32)
        rhs_m = np.zeros((NP * K, W), np.float32)
        idx_m = np.full(((NP + 1) // 2, P, 2 * W), N * N, np.int64)
        for p, (_, chlist) in enumerate(per_core[c]):
            off = 0
            for band, (R, s0, w) in enumerate(chlist):
                rows = cs[R * P : (R + 1) * P]
                bmin, bmax = rows.min(0), rows.max(0)
                tR = ((bmin + bmax) * np.float32(0.5)).astype(np.float32)
                rl = (rows - tR).astype(np.float32)
                cand = cands[R][s0 : s0 + w]
                cl = (cs[cand] - tR).astype(np.float32)
                ri = ((rl[:, 0] * rl[:, 0] + rl[:, 1] * rl[:, 1]) + rl[:, 2] * rl[:, 2]).astype(np.float32)
                rj = ((cl[:, 0] * cl[:, 0] + cl[:, 1] * cl[:, 1]) + cl[:, 2] * cl[:, 2]).astype(np.float32)
                kb = p * K + band * KB
                rcol = slice(off, off + w)
                NC = len(KEEP)
                for ci in range(3):
                    rs = _split3(rl[:, ci].copy())
                    csp = _split3(cl[:, ci].copy())
                    for i, (a, bb) in enumerate(KEEP):
                        wa = (np.float32(-2.0) * rs[a]).astype(bf).astype(np.float32)
                        row = kb + ci * NC + i
                        wts_m[row, :] = wa
                        rhs_m[row, rcol] = csp[bb]
                for a, sp in enumerate(_split3(ri.copy())):
                    wts_m[kb + 3 * NC + a, :] = sp
                    rhs_m[kb + 3 * NC + a, rcol] = 1.0
                for bb, sp in enumerate(_split3(rj.copy())):
                    wts_m[kb + 3 * NC + 3 + bb, :] = 1.0
                    rhs_m[kb + 3 * NC + 3 + bb, rcol] = sp
                # scatter indices: orig (hi, lo) pair -> tril slot; self -> scratch
                ro = pi[R * P : (R + 1) * P]
                co = pi[cand]
                hi = np.maximum(ro[:, None], co[None, :])
                lo = np.minimum(ro[:, None], co[None, :])
                flat = hi * N + lo
                flat[ro[:, None] == co[None, :]] = N * N
                cb = (p % 2) * W
                idx_m[p // 2, :, cb + off : cb + off + w] = flat
                off += w
        inp_m = np.concatenate([wts_m, rhs_m], axis=1)  # [NP*K, P+W]
        in_maps.append({"inp": np.ascontiguousarray(inp_m).astype(bf)})
        idx_maps.append(idx_m)
    with _lock:
        _cache["cuthi"] = float(cuthi)
    return in_maps, idx_maps, NP


def _prepare_inputs(species, coordinates, cutoff):
    in_maps, idx_maps, NP = _prepare(species, coordinates, cutoff)
    return in_maps


def _run(in_maps, trace=False):
    from concourse import bass_utils

    NP = in_maps[0]["inp"].shape[0] // K
    with _lock:
        cuthi = _cache["cuthi"]
    nc = _get_program(NP, cuthi)
    return bass_utils.run_bass_kernel_spmd(
        nc, in_maps, core_ids=list(range(NCORES)), trace=trace
    )


def _assemble(results, idx_maps):
    full = np.zeros(N * N + 1, np.float32)
    for c in range(NCORES):
        vals = results[c]["out"].astype(np.float32)
        full[idx_maps[c].ravel()] = vals.ravel()
    return full[: N * N].reshape(N, N)


def kernel(species, coordinates, cutoff):
    in_maps, idx_maps, NP = _prepare(species, coordinates, cutoff)
    res = _run(in_maps)
    return _assemble(res.results, idx_maps)


# revision 20
# speedup vs baseline: 1.0853x; 1.0511x over previous
"""Cell-list pairwise distance screen (CellList) for 8 Trainium2 NeuronCores.

Computes the masked dense [N, N] lower-triangular distance matrix:
  out[i, j] = sqrt(|c_i - c_j|^2)  if  j < i, both species valid, d2 <= cutoff^2
            = 0                    otherwise

Strategy (block-sparse + single-matmul d2):
  - Host sorts atoms along a Hilbert curve over 2.5 A cells -> 48 row blocks
    of 128 spatially-compact atoms.  For each row block R it gathers the
    candidate columns {j : dist(j, bbox_R) <= cutoff, block(j) <= R} -- a
    conservative superset of all pairs, deduplicated at block level (each
    cross-block pair appears in exactly one list; own-block pairs appear in
    both orientations of the self tile and scatter to the same output slot).
  - d2 is produced by ONE tensor-engine matmul per 512-col piece:
      d2[i,j] = ri + rj - 2*ci.cj
    expanded over exact 3-way bf16 splits of the per-block-translated
    coordinates (local coords ~ +-13 A, so f32 cancellation error ~1e-5,
    far below the ~3e-5 spacing of d2 values near cutoff^2; measured 0 mask
    flips vs the f32 reference on the target data).  33 contraction rows per
    block: 27 split cross products + 3 ri splits (x ones) + 3 rj splits.
  - Up to 3 row-block segments pack into one 512-col piece as block-diagonal
    bands (K = 99): a column's rhs rows are zero outside its own band, so
    each column only accumulates its own block's terms.  Candidate lists
    split freely across pieces/cores, so the 8 cores get an equal number of
    nearly-full pieces.
  - PSUM then holds d2 directly: one DVE bandpass (select t in (1e-3,
    cutoff^2], else 0) -> one ACT sqrt to fp16 -> DMA out.  Host scatters
    the compacted fp16 values into the full [N, N] f32 zero matrix.
  - All DRAM tensors are laid out so every per-piece DMA is one contiguous
    block (cheap descriptors): rhs [NP*K, W], wts [NP*K, P], out [NP*P, W].
"""

import threading

import numpy as np

N = 6144
P = 128
NCORES = 8
W = 512  # piece width (one PSUM bank)
KB = 24  # contraction rows per band
BANDS = 3  # bands (row-block segments) per piece
K = KB * BANDS  # 72
# cross-product split pairs kept (row-split a, col-split b); the dropped
# (m,l)/(l,m)/(l,l) terms are < 1e-5 in d2 -- measured 0 mask flips
KEEP = [(0, 0), (0, 1), (0, 2), (1, 0), (1, 1), (2, 0)]

_lock = threading.Lock()
_cache: dict = {}


def _register_ops():
    """Register the fused DVE bandpass op at runtime (visible to table-gen)."""
    import concourse.dve_ops as dve_ops
    from concourse.dve_spec import (
        C0,
        C1,
        Spec,
        Src0,
        Zero,
        _has_src1,
        lower,
        select,
    )
    from concourse.dve_uop import DveOpSpec

    def make(name, body, ref):
        for op in dve_ops.OPS:
            if op.name == name:
                return op
        spec = Spec(body=body, reference=ref)
        row = 1 + len(dve_ops.OPS)
        assert row < 0x20
        shas = {}
        for ver in ("v3", "v4"):
            uops = lower(spec, ver=ver)
            shas[ver] = DveOpSpec(
                name=name, opcode=row, uops=uops, rd1_en=_has_src1(spec)
            ).sha(ver)
        op = dve_ops.DveOp(name, spec, subdim=False, uops_sha=shas)
        dve_ops._SUB_OPCODE_FOR_NAME[name] = row
        dve_ops.OPS.append(op)
        dve_ops.CUSTOM_DVE_SPECS[name] = spec
        return op

    # out = (s0 < in0 < s1) ? in0 : 0
    def band_ref(in0, in1, s0, s1, imm2):
        t = in0.astype(np.float32)
        keep = (t > s0) & (t < s1)
        return np.where(keep, t, 0.0).astype(np.float32)

    bandpass = make(
        "BANDPASS_ANT",
        select((Src0 > C0) & (Src0 < C1), Src0, Zero),
        band_ref,
    )
    return bandpass


def _build_program(NP, cuthi):
    import concourse.bacc as bacc
    import concourse.mybir as mybir
    import concourse.tile as tile

    bandpass = _register_ops()

    nc = bacc.Bacc("TRN2", target_bir_lowering=False, debug=False, num_devices=NCORES)
    f32 = mybir.dt.float32
    f16 = mybir.dt.float16
    bf16 = mybir.dt.bfloat16
    PW = P + W  # fused [wts | rhs] block width per piece

    NSP = (NP + 1) // 2  # 1024-col super-pieces for DVE/ACT/out-DMA
    inp = nc.dram_tensor("inp", [NP * K, PW], bf16, kind="ExternalInput")
    out = nc.dram_tensor("out", [NSP * P, 2 * W], f16, kind="ExternalOutput")

    with tile.TileContext(nc) as tc:
        with (
            tc.tile_pool(name="const", bufs=1) as cpool,
            tc.tile_pool(name="work", bufs=3) as wpool,
            tc.tile_pool(name="outp", bufs=3) as spool,
            tc.tile_pool(name="psx", bufs=3, space="PSUM") as ppx,
        ):
            cc_t = cpool.tile([P, 2], f32, tag="cc")
            inp_t = [
                cpool.tile([K, PW], bf16, tag=f"inp{p}", name=f"inp{p}")
                for p in range(NP)
            ]
            warm_t = cpool.tile([P, 2], f32, tag="warm")

            # one contiguous DMA per piece over all three hw issue queues;
            # per-DMA completion latency is ~2us, so earliest-needed first.
            # piece 0 is split across both fast queues to start compute
            # as early as possible.
            H = K // 2
            nc.sync.dma_start(inp_t[0][0:H, :], inp[0:H, :])
            nc.gpsimd.dma_start(inp_t[0][H:K, :], inp[H:K, :])
            queues = [None, nc.sync, nc.scalar, nc.gpsimd, nc.sync]
            for p in range(1, NP):
                queues[p % 5].dma_start(inp_t[p][:], inp[p * K : (p + 1) * K, :])

            # bandpass consts via memset on the otherwise-idle vector queue
            # (no DMA); cuthi is baked per build
            nc.vector.memset(cc_t[:, 0:1], 1e-3)
            nc.vector.memset(cc_t[:, 1:2], float(cuthi))

            # pull the ACT sqrt table in
            nc.vector.memset(warm_t[:, 0:1], 1.0)
            nc.scalar.sqrt(warm_t[:, 1:2], warm_t[:, 0:1])

            for sp in range(NSP):
                pieces = [p for p in (2 * sp, 2 * sp + 1) if p < NP]
                w = len(pieces) * W
                t = ppx.tile([P, 2 * W], f32, tag="t")
                for h, p in enumerate(pieces):
                    nc.tensor.matmul(
                        t[:, h * W : (h + 1) * W],
                        inp_t[p][:, 0:P],
                        inp_t[p][:, P:PW],
                        start=True,
                        stop=True,
                    )
                v = wpool.tile([P, 2 * W], f16, tag="v")
                nc.vector._custom_dve(
                    bandpass,
                    out=v[:, 0:w],
                    in0=t[:, 0:w],
                    s0=cc_t[:, 0:1],
                    s1=cc_t[:, 1:2],
                )
                s = spool.tile([P, 2 * W], f16, tag="s")
                nc.scalar.sqrt(s[:, 0:w], v[:, 0:w])
                q = nc.gpsimd if sp % 2 == 0 else nc.sync
                q.dma_start(out[sp * P : (sp + 1) * P, 0:w], s[:, 0:w])

    nc.compile()
    return nc


def _get_program(NP, cuthi):
    with _lock:
        key = f"nc{NP}-{float(cuthi)}"
        if key not in _cache:
            _cache[key] = _build_program(NP, cuthi)
    return _cache[key]


def _hilbert_sort(coords):
    """Atom permutation along a Hilbert curve over a 16^3 grid."""
    lo = coords.min(0)
    ext = np.maximum(coords.max(0) - lo, 1e-6)
    cell = np.clip((coords - lo) / ext * 16.0, 0, 15.999).astype(np.int64)
    X = cell.T.astype(np.uint64).copy()
    n, bits = 3, 4
    M = np.uint64(1) << np.uint64(bits - 1)
    Q = M
    while Q > np.uint64(1):
        Pm = Q - np.uint64(1)
        for i in range(n):
            hi = (X[i] & Q) != 0
            X[0] = np.where(hi, X[0] ^ Pm, X[0])
            t = (X[0] ^ X[i]) & Pm
            X[0] = np.where(hi, X[0], X[0] ^ t)
            X[i] = np.where(hi, X[i], X[i] ^ t)
        Q >>= np.uint64(1)
    for i in range(1, n):
        X[i] ^= X[i - 1]
    t = np.zeros(len(cell), np.uint64)
    Q = M
    while Q > np.uint64(1):
        t = np.where((X[n - 1] & Q) != 0, t ^ (Q - np.uint64(1)), t)
        Q >>= np.uint64(1)
    for i in range(n):
        X[i] ^= t
    key = np.zeros(len(cell), np.int64)
    for b in range(bits):
        for i in range(3):
            key |= np.int64(((X[i] >> np.uint64(b)) & np.uint64(1)).astype(np.int64)) << np.int64(
                3 * b + (2 - i)
            )
    return np.argsort(key, kind="stable")


def _split3(v32):
    """Exact 3-way bf16 split: v32 == hi + mid + lo (as f32 sums)."""
    import ml_dtypes

    bf = ml_dtypes.bfloat16
    hi = v32.astype(bf)
    r1 = (v32 - hi.astype(np.float32)).astype(np.float32)
    mid = r1.astype(bf)
    r2 = (r1 - mid.astype(np.float32)).astype(np.float32)
    lo = r2.astype(bf)
    recon = (
        hi.astype(np.float32) + mid.astype(np.float32) + lo.astype(np.float32)
    ).astype(np.float32)
    assert np.array_equal(recon, v32), "bf16 3-way split not exact"
    return hi.astype(np.float32), mid.astype(np.float32), lo.astype(np.float32)


def _prepare(species, coordinates, cutoff):
    """Build per-core in_maps plus host-side scatter indices."""
    import ml_dtypes

    bf = ml_dtypes.bfloat16
    coords = np.asarray(coordinates, dtype=np.float32).reshape(-1, 3).copy()
    n = coords.shape[0]
    assert n == N and n % P == 0, coords.shape
    valid = np.asarray(species).reshape(-1) >= 0
    if not valid.all():
        bad = np.where(~valid)[0]
        far = float(coords[valid].max()) if valid.any() else 0.0
        coords[bad] = (far + 20.0 + 10.0 * np.arange(len(bad), dtype=np.float32))[
            :, None
        ]

    cutf = float(cutoff)
    cut2 = np.float32(cutf) * np.float32(cutf)
    cuthi = np.nextafter(cut2, np.float32(np.inf), dtype=np.float32)
    prune2 = (cutf + 1e-3) ** 2  # conservative host-side pruning radius

    pi = _hilbert_sort(coords)
    cs = coords[pi].astype(np.float32)
    NB = n // P
    blk = np.arange(n) // P

    # candidate columns per row block, deduped at block level
    cands = []
    for R in range(NB):
        rows = cs[R * P : (R + 1) * P]
        bmin, bmax = rows.min(0), rows.max(0)
        d = np.maximum(0, np.maximum(bmin[None, :] - cs, cs - bmax[None, :]))
        cand = np.where(((d * d).sum(1) <= prune2) & (blk <= R))[0]
        cands.append(cand)

    # greedy pour, largest list first, splitting freely at piece boundaries;
    # each piece holds <= W cols and <= BANDS row-block segments
    order = sorted(range(NB), key=lambda R: -len(cands[R]))
    bins = []  # [space_left, [(R, start, width), ...]]
    cur = None
    for R in order:
        left = len(cands[R])
        s0 = 0
        while left > 0:
            if cur is None or cur[0] == 0 or len(cur[1]) == BANDS:
                bins.append([W, []])
                cur = bins[-1]
            take = min(left, cur[0])
            cur[1].append((R, s0, take))
            cur[0] -= take
            s0 += take
            left -= take
    nbins = len(bins)
    NP = max(1, -(-nbins // NCORES))

    # assign bins to cores round-robin by size
    border = sorted(range(nbins), key=lambda i: -(W - bins[i][0]))
    per_core = [[] for _ in range(NCORES)]
    for i, b in enumerate(border):
        per_core[i % NCORES].append(bins[b])

    in_maps = []
    idx_maps = []
    for c in range(NCORES):
        wts_m = np.zeros((NP * K, P), np.float32)
        rhs_m = np.zeros((NP * K, W), np.float32)
        idx_m = np.full(((NP + 1) // 2, P, 2 * W), N * N, np.int64)
        for p, (_, chlist) in enumerate(per_core[c]):
            off = 0
            for band, (R, s0, w) in enumerate(chlist):
                rows = cs[R * P : (R + 1) * P]
                bmin, bmax = rows.min(0), rows.max(0)
                tR = ((bmin + bmax) * np.float32(0.5)).astype(np.float32)
                rl = (rows - tR).astype(np.float32)
                cand = cands[R][s0 : s0 + w]
                cl = (cs[cand] - tR).astype(np.float32)
                ri = ((rl[:, 0] * rl[:, 0] + rl[:, 1] * rl[:, 1]) + rl[:, 2] * rl[:, 2]).astype(np.float32)
                rj = ((cl[:, 0] * cl[:, 0] + cl[:, 1] * cl[:, 1]) + cl[:, 2] * cl[:, 2]).astype(np.float32)
                kb = p * K + band * KB
                rcol = slice(off, off + w)
                NC = len(KEEP)
                for ci in range(3):
                    rs = _split3(rl[:, ci].copy())
                    csp = _split3(cl[:, ci].copy())
                    for i, (a, bb) in enumerate(KEEP):
                        wa = (np.float32(-2.0) * rs[a]).astype(bf).astype(np.float32)
                        row = kb + ci * NC + i
                        wts_m[row, :] = wa
                        rhs_m[row, rcol] = csp[bb]
                for a, sp in enumerate(_split3(ri.copy())):
                    wts_m[kb + 3 * NC + a, :] = sp
                    rhs_m[kb + 3 * NC + a, rcol] = 1.0
                for bb, sp in enumerate(_split3(rj.copy())):
                    wts_m[kb + 3 * NC + 3 + bb, :] = 1.0
                    rhs_m[kb + 3 * NC + 3 + bb, rcol] = sp
                # scatter indices: orig (hi, lo) pair -> tril slot; self -> scratch
                ro = pi[R * P : (R + 1) * P]
                co = pi[cand]
                hi = np.maximum(ro[:, None], co[None, :])
                lo = np.minimum(ro[:, None], co[None, :])
                flat = hi * N + lo
                flat[ro[:, None] == co[None, :]] = N * N
                cb = (p % 2) * W
                idx_m[p // 2, :, cb + off : cb + off + w] = flat
                off += w
        inp_m = np.concatenate([wts_m, rhs_m], axis=1)  # [NP*K, P+W]
        in_maps.append({"inp": np.ascontiguousarray(inp_m).astype(bf)})
        idx_maps.append(idx_m)
    with _lock:
        _cache["cuthi"] = float(cuthi)
    return in_maps, idx_maps, NP


def _prepare_inputs(species, coordinates, cutoff):
    in_maps, idx_maps, NP = _prepare(species, coordinates, cutoff)
    return in_maps


def _run(in_maps, trace=False):
    from concourse import bass_utils

    NP = in_maps[0]["inp"].shape[0] // K
    with _lock:
        cuthi = _cache["cuthi"]
    nc = _get_program(NP, cuthi)
    return bass_utils.run_bass_kernel_spmd(
        nc, in_maps, core_ids=list(range(NCORES)), trace=trace
    )


def _assemble(results, idx_maps):
    full = np.zeros(N * N + 1, np.float32)
    for c in range(NCORES):
        vals = results[c]["out"].astype(np.float32)
        full[idx_maps[c].ravel()] = vals.ravel()
    return full[: N * N].reshape(N, N)


def kernel(species, coordinates, cutoff):
    in_maps, idx_maps, NP = _prepare(species, coordinates, cutoff)
    res = _run(in_maps)
    return _assemble(res.results, idx_maps)
